# revision 20
# baseline (speedup 1.0000x reference)
"""Trainium2 Bass kernel for the ConvolutionalKAN problem.

Math: the KAN conv
    out[b,o,y,x] = sum_{j,kk,l,m} phi_m(11*inp[b,j,y+kk,x+l]) * coeff[o,j,kk,l,m]
with phi_m the degree-3 B-spline basis on uniform knots linspace(0,1,12)
is rewritten via the truncated-power identity
    phi_m(t) = sum_{k=0..4} (-1)^k C(4,k)/6 * relu(t - m - k)^3
so with g_s = relu(11*x - s)^3 (s = 0..11) the whole thing is a VALID 3x3
conv over 64*12 = 768 input channels with host-folded weights
    chat[o,j,kk,l,s] = sum_{m+k=s} (-1)^k C(4,k)/6 * coeff[o,j,kk,l,m].

Sharding: data-parallel over batch, 2 images per core on 8 cores.
On-chip: g is computed strip-wise (18 input rows) on ACT/DVE/GpSimd and
consumed by TensorE as a chain of 54 accumulating matmuls per 8-row
output group (contraction 6 c-tiles x 9 taps), fp32 data issued as
float32r for full-rate PE streaming.
"""

import os
import sys

import numpy as np

for _p in ("/root/.axon_site/_ro/trn_rl_repo", "/opt/trn_rl_repo"):
    if os.path.isdir(_p) and _p not in sys.path:
        sys.path.append(_p)

B_FULL = 16
N_CORES = 8
B_SHARD = B_FULL // N_CORES
CIN = 64
COUT = 64
H = 64
W = 64
KS = 3
NB = 8
NS = 8  # B-spline basis channels per input channel
HO = H - KS + 1  # 62
WO = W - KS + 1  # 62
NQ = (CIN * NS) // 128  # 6 contraction tiles of 128
TAPS = KS * KS

# output row strips: (y0, n_input_rows, per-group output rows)
STRIPS = [(0, 18, (8, 8)), (16, 18, (8, 8)), (32, 18, (8, 8)), (48, 16, (8, 6))]

MM_DTYPE_ENV = os.environ.get("KAN_MM_DTYPE", "float32r")
C2 = 4.0 ** (1.0 / 3.0)


SPLIT = 5.5  # value-domain split point for the dual-rail clamped basis


def _fold_coeff(coeff: np.ndarray):
    """coeff [COUT, CIN, KS, KS, NB] -> (W_host [512, 576] f32, obias [128,1]).

    Channels fed to the matmul are 6*phi_m(t) = relu(2-u)^3 - (c2*relu(1-u))^3
    with u = |t-(m+2)|, so the folded weights are just coeff/6 in layout
    W_host[m*64 + j, (kk*3+l)*64 + o].
    """
    w = (coeff.astype(np.float64).transpose(4, 1, 2, 3, 0) / 6.0).reshape(
        NS * CIN, TAPS * COUT)
    return (np.ascontiguousarray(w, dtype=np.float32),
            np.zeros((128, 1), dtype=np.float32))


def _build_bass():
    import concourse.bacc as bacc
    import concourse.mybir as mybir
    import concourse.tile as tile

    f32 = mybir.dt.float32
    mm_dt = getattr(mybir.dt, MM_DTYPE_ENV)
    use_f32r = mm_dt != f32
    AF = mybir.ActivationFunctionType

    nc = bacc.Bacc("TRN2", target_bir_lowering=False, debug=False,
                   num_devices=N_CORES)
    x_d = nc.dram_tensor("x", [B_SHARD, CIN, H, W], f32, kind="ExternalInput").ap()
    w_d = nc.dram_tensor("w", [NS * CIN, TAPS * COUT], f32, kind="ExternalInput").ap()
    b_d = nc.dram_tensor("btbl", [128, NQ + 2], f32, kind="ExternalInput").ap()
    ob_d = nc.dram_tensor("obias", [128, 1], f32, kind="ExternalInput").ap()
    out_d = nc.dram_tensor("out", [B_SHARD, COUT, HO, WO], f32,
                           kind="ExternalOutput").ap()

    col_tile = os.environ.get("KAN_COLTILE", "0") == "1"
    wsplit = os.environ.get("KAN_WSPLIT", "0") == "1"
    w_dt = f32 if os.environ.get("KAN_W_EXACT", "0") == "1" else mm_dt

    with tile.TileContext(nc) as tc:
        from contextlib import ExitStack

        with ExitStack() as ctx:
            wpool = ctx.enter_context(tc.tile_pool(name="w", bufs=NQ))
            cpool = ctx.enter_context(tc.tile_pool(name="const", bufs=1))
            xpool = ctx.enter_context(tc.tile_pool(name="x", bufs=3))
            gpool = ctx.enter_context(tc.tile_pool(name="g", bufs=3 * NQ))
            rpool = ctx.enter_context(tc.tile_pool(name="r", bufs=2))
            spool = ctx.enter_context(tc.tile_pool(name="sq", bufs=2))
            opool = ctx.enter_context(tc.tile_pool(name="o", bufs=3))
            ppool = ctx.enter_context(
                tc.tile_pool(name="ps", bufs=2, space="PSUM"))

            bt = cpool.tile([128, NQ + 2], f32)
            nc.sync.dma_start(bt[:], b_d[:])
            obt = cpool.tile([128, 1], f32, tag="obias")
            nc.sync.dma_start(obt[:], ob_d[:])
            wts = []
            for q in range(NQ):
                wt = wpool.tile([128, TAPS * COUT], f32, tag="wstage")
                nc.sync.dma_start(wt[:], w_d[q * 128:(q + 1) * 128, :])
                if wsplit:
                    # [w_hi | w_lo] per tap: wc[:, tap*128:+64] = f32r(w),
                    # [+64:+128] = f32r(w - w_hi); stationary M=128 costs no
                    # extra PE time and makes weights ~24-bit effective.
                    wc = wpool.tile([128, 2 * TAPS * COUT], mm_dt, tag="wc")
                    wcv = wc[:].rearrange("p (t h o) -> p t h o", h=2, o=COUT)
                    wtv = wt[:].rearrange("p (t o) -> p t o", o=COUT)
                    nc.vector.tensor_copy(wcv[:, :, 0, :], wtv)
                    nc.vector.tensor_sub(wcv[:, :, 1, :], wtv, wcv[:, :, 0, :])
                    wts.append(wc)
                elif w_dt != f32:
                    wr = wpool.tile([128, TAPS * COUT], w_dt, tag="wr")
                    nc.vector.tensor_copy(wr[:], wt[:])
                    wts.append(wr)
                else:
                    wts.append(wt)

            alu = mybir.AluOpType
            NPIX = 18 * W
            for b in range(B_SHARD):
                for (y0, nin, groups) in STRIPS:
                    npx = nin * W
                    xt = xpool.tile([128, NPIX], f32)
                    src = x_d[b, :, y0:y0 + nin, :]
                    nc.sync.dma_start(
                        xt[0:64].rearrange("p (r c) -> p r c", c=W)[:, :nin, :],
                        src)
                    nc.sync.dma_start(
                        xt[64:128].rearrange("p (r c) -> p r c", c=W)[:, :nin, :],
                        src)
                    gts = []
                    for q in range(NQ):
                        # u = |11x - (m+2)|; 6*phi = relu(2-u)^3 - (c2*relu(1-u))^3
                        g = gpool.tile([128, NPIX], mm_dt)
                        u = rpool.tile([128, NPIX], f32, tag="u")
                        a = rpool.tile([128, NPIX], f32, tag="a")
                        bb = rpool.tile([128, NPIX], f32, tag="b")
                        sa = spool.tile([128, NPIX], f32, tag="sa")
                        sb = spool.tile([128, NPIX], f32, tag="sb")
                        a3 = spool.tile([128, NPIX], f32, tag="a3")
                        bias = bt[:, q:q + 1]
                        nc.scalar.activation(u[:, :npx], xt[:, :npx],
                                             AF.Abs, bias=bias, scale=11.0)
                        nc.scalar.activation(a[:, :npx], u[:, :npx],
                                             AF.Relu, bias=bt[:, NQ:NQ + 1],
                                             scale=-1.0)
                        nc.scalar.activation(bb[:, :npx], u[:, :npx],
                                             AF.Relu, bias=bt[:, NQ + 1:NQ + 2],
                                             scale=-C2)
                        if q < 3:
                            nc.scalar.activation(sa[:, :npx], u[:, :npx],
                                                 AF.Square,
                                                 bias=bt[:, NQ:NQ + 1],
                                                 scale=-1.0)
                        else:
                            nc.vector.tensor_mul(sa[:, :npx], a[:, :npx],
                                                 a[:, :npx])
                        nc.gpsimd.tensor_mul(sb[:, :npx], bb[:, :npx],
                                             bb[:, :npx])
                        nc.vector.tensor_mul(a3[:, :npx], sa[:, :npx],
                                             a[:, :npx])
                        nc.vector.tensor_mul(sb[:, :npx], sb[:, :npx],
                                             bb[:, :npx])
                        nc.vector.tensor_sub(g[:, :npx], a3[:, :npx],
                                             sb[:, :npx])
                        gts.append(g)

                    gvs = [g[:].rearrange("p (r c) -> p r c", c=W) for g in gts]
                    n_mm = NQ * TAPS
                    if wsplit:
                        for grp, nr in enumerate(groups):
                            ps = ppool.tile([128, 8, WO], f32)
                            i_mm = 0
                            for q in range(NQ):
                                for kk in range(KS):
                                    for l in range(KS):
                                        r0 = 8 * grp + kk
                                        rhs = gvs[q][:, r0:r0 + nr, l:l + WO]
                                        tap = kk * KS + l
                                        lhsT = wts[q][:, tap * 2 * COUT:
                                                      (tap + 1) * 2 * COUT]
                                        nc.tensor.matmul(
                                            ps[:, :nr, :], lhsT, rhs,
                                            start=(i_mm == 0),
                                            stop=(i_mm == n_mm - 1),
                                        )
                                        i_mm += 1
                            ot = opool.tile([64, 8, WO], f32)
                            nc.scalar.activation(ot[:, :nr, :], ps[0:64, :nr, :],
                                                 AF.Identity, bias=obt[0:64, 0:1],
                                                 scale=1.0)
                            nc.vector.scalar_tensor_tensor(
                                ot[:, :nr, :], ps[64:128, :nr, :], 1.0,
                                ot[:, :nr, :], op0=alu.mult, op1=alu.add)
                            nc.sync.dma_start(
                                out_d[b, :, y0 + 8 * grp:y0 + 8 * grp + nr, :],
                                ot[:, :nr, :])
                    elif col_tile:
                        # both output groups in one chain, 2 col-tiles
                        ps = ppool.tile([128, 8, WO], f32)
                        i_mm = 0
                        for q in range(NQ):
                            for kk in range(KS):
                                for l in range(KS):
                                    lhsT = wts[q][:, (kk * KS + l) * COUT:
                                                  (kk * KS + l + 1) * COUT]
                                    for grp, nr in enumerate(groups):
                                        r0 = 8 * grp + kk
                                        rhs = gvs[q][:, r0:r0 + nr, l:l + WO]
                                        nc.tensor.matmul(
                                            ps[64 * grp:64 * grp + 64, :nr, :],
                                            lhsT, rhs,
                                            start=(i_mm == 0),
                                            stop=(i_mm == n_mm - 1),
                                            tile_position=(0, 64 * grp),
                                        )
                                    i_mm += 1
                        ot = opool.tile([128, 8, WO], f32)
                        for grp, nr in enumerate(groups):
                            sl = slice(64 * grp, 64 * grp + 64)
                            nc.scalar.activation(ot[sl, :nr, :], ps[sl, :nr, :],
                                                 AF.Identity, bias=obt[sl, 0:1],
                                                 scale=1.0)
                            nc.sync.dma_start(
                                out_d[b, :, y0 + 8 * grp:y0 + 8 * grp + nr, :],
                                ot[sl, :nr, :])
                    else:
                        for grp, nr in enumerate(groups):
                            ps = ppool.tile([64, 8, WO], f32)
                            i_mm = 0
                            for q in range(NQ):
                                for kk in range(KS):
                                    for l in range(KS):
                                        r0 = 8 * grp + kk
                                        rhs = gvs[q][:, r0:r0 + nr, l:l + WO]
                                        lhsT = wts[q][:, (kk * KS + l) * COUT:
                                                      (kk * KS + l + 1) * COUT]
                                        nc.tensor.matmul(
                                            ps[:, :nr, :], lhsT, rhs,
                                            start=(i_mm == 0),
                                            stop=(i_mm == n_mm - 1),
                                        )
                                        i_mm += 1
                            ot = opool.tile([64, 8, WO], f32)
                            nc.scalar.activation(ot[:, :nr, :], ps[:, :nr, :],
                                                 AF.Identity, bias=obt[0:64, 0:1],
                                                 scale=1.0)
                            nc.sync.dma_start(
                                out_d[b, :, y0 + 8 * grp:y0 + 8 * grp + nr, :],
                                ot[:, :nr, :])

    nc.compile()
    return nc


def _maybe_install_profile_shim():
    """Allow trace=True/BASS_TRACE under axon even though this image lacks
    antenv.axon_hooks; degrade silently if anything is missing."""
    import types

    if "antenv.axon_hooks" in sys.modules:
        return
    try:
        from trn_agent_boot.trn_boot import _ntff_profile_via_ctypes

        hook = _ntff_profile_via_ctypes("/opt/axon/libaxon_pjrt.so")
        if hook is None:
            return
        mod = types.ModuleType("antenv.axon_hooks")
        mod.get_axon_ntff_profile_hook = lambda: hook
        mod.set_axon_ntff_profile_hook = lambda h: None
        sys.modules["antenv.axon_hooks"] = mod
        from concourse import bass_utils

        bass_utils.upload_artifacts = lambda tmpdir: f"local:{tmpdir}"
    except Exception:
        pass


_LAST_RESULTS = None


def kernel(x: np.ndarray, coeff: np.ndarray) -> np.ndarray:
    global _LAST_RESULTS
    from concourse import bass_utils

    _maybe_install_profile_shim()

    x = np.ascontiguousarray(np.asarray(x), dtype=np.float32)
    coeff = np.asarray(coeff)
    assert x.shape == (B_FULL, CIN, H, W), x.shape

    w_host, obias = _fold_coeff(coeff)
    btbl = np.zeros((128, NQ + 2), dtype=np.float32)
    for p in range(128):
        for q in range(NQ):
            m = 2 * q + (1 if p >= 64 else 0)
            btbl[p, q] = -float(m + 2)
    btbl[:, NQ] = 2.0
    btbl[:, NQ + 1] = C2

    nc = _build_bass()

    in_maps = []
    for i in range(N_CORES):
        in_maps.append({
            "x": np.ascontiguousarray(x[i * B_SHARD:(i + 1) * B_SHARD]),
            "w": w_host,
            "btbl": btbl,
            "obias": obias,
        })

    res = bass_utils.run_bass_kernel_spmd(
        nc, in_maps, core_ids=list(range(N_CORES)),
        trace=bool(os.environ.get("KAN_TRACE")),
    )
    _LAST_RESULTS = res

    out = np.concatenate([res.results[i]["out"] for i in range(N_CORES)], axis=0)
    return out.astype(np.float32, copy=False)
